# revision 5
# baseline (speedup 1.0000x reference)
"""Trainium2 Bass kernel for nn_CgTransform (L=7, T=128, 8 NeuronCores).

Math: for each (l1,l2) block pair and each kept output row k=(l,m):
    OUT_k[t1,t2] = sum_{i,j} C[k, l1^2+i, l2^2+j] * clms[l1^2+i, t1] * clms[l2^2+j, t2]
                 = A1^T (C_k A2).

Symmetry (verified numerically): CG coefficient symmetry gives
    OUT_{(l2,l1),k} = (-1)^{l1+l2-l} * OUT_{(l1,l2),k}^T,
so the device computes only the 36 pairs with l1<=l2 (1378 of 2416 output
rows); the host mirrors the rest by transposing [T,T] blocks.

Device restructuring (per output row k = one "slot"):
  stage 1:  W_k[i, t2] = sum_j C_k[i,j] * clms[l2^2+j, t2]
            -> dense matmul W_chunk = S_chunk^T.T @ clms per 128-partition
               chunk; 4 slots per chunk at 32-partition bands.
  stage 2:  OUT_k[t1,t2] = sum_i A1pad[i,t1] * W_k[i,t2]
            -> slots are grouped into CELLS of 4 consecutive k of the SAME
               pair spread over the 4 chunks of a group, so one N=512 matmul
               (lhsT = padded A1 at band 32b, rhs = the group's W band rows)
               computes 4 slots. tile_position=(32b, 0); each cell gets its
               own PSUM bank (HW: different tile_positions must not share a
               bank).

Sharding: 384 padded cells split 48 per core (identical SPMD program,
per-core S/A1 data). Output layout per core: [t1, group, band, j, t2]; host
transposes to [k, t1, t2] and reassembles the 8-tuple.

Env KERNEL_F32R=1: stage-2 runs in float32r (~4x faster matmul streaming,
~1e-4 rel err instead of ~2e-7).
"""
import os
import sys
import types

if "/opt/trn_rl_repo" not in sys.path:
    sys.path.insert(0, "/opt/trn_rl_repo")

import numpy as np

L = 7
LSIZE = (L + 1) ** 2          # 64
T = 128
NCORES = 8
SLOT_P = 32                   # partitions per slot band
BANDS = 4                     # bands (cells) per group
GROUP_CHUNKS = 4              # chunks per group == slots per cell

# ------------------------------------------------------------ pair tables
PAIRS = []                    # full table: (l1, l2, lo, hi, rows, row_offset)
_off = 0
for _l1 in range(L + 1):
    for _l2 in range(L + 1):
        _lo, _hi = abs(_l1 - _l2), min(_l1 + _l2, L)
        _rows = (_hi + 1) ** 2 - _lo * _lo
        PAIRS.append((_l1, _l2, _lo, _hi, _rows, _off))
        _off += _rows
TOTAL_ROWS = _off             # 2416

UP_PAIRS = [i for i, p in enumerate(PAIRS) if p[0] <= p[1]]   # 36 pair idxs

# global cell list: (full_pair_idx, k_base); one pair per cell, 4 k's
CELL_TABLE = []
for _pi in UP_PAIRS:
    _l1, _l2, _lo, _hi, _rows, _po = PAIRS[_pi]
    for _kb in range(_lo * _lo, (_hi + 1) ** 2, GROUP_CHUNKS):
        CELL_TABLE.append((_pi, _kb))
N_CELLS = len(CELL_TABLE)     # 356

CELLS_PER_CORE = -(-(-(-N_CELLS // NCORES)) // BANDS) * BANDS  # 48
GROUPS = CELLS_PER_CORE // BANDS       # 12
CHUNKS = GROUPS * GROUP_CHUNKS         # 48
SLOTS = CELLS_PER_CORE * GROUP_CHUNKS  # 192 per core
ST_COLS = CHUNKS * T                   # 6144
A1_COLS = GROUPS * T                   # 1536
OUT_COLS = SLOTS * T                   # 24576

_NC = None
LAST_EXEC_TIME_NS = None


def _build_nc(use_f32r):
    import concourse.bacc as bacc
    import concourse.mybir as mybir
    import concourse.tile as tile

    F32 = mybir.dt.float32
    WDT = mybir.dt.float32r if use_f32r else F32
    nc = bacc.Bacc("TRN2", target_bir_lowering=False, debug=False, num_devices=1)
    d_clms = nc.dram_tensor("clms", [LSIZE, T], WDT, kind="ExternalInput")
    d_st = nc.dram_tensor("s_t", [LSIZE, ST_COLS], WDT, kind="ExternalInput")
    d_a1 = nc.dram_tensor("a1s", [128, A1_COLS], WDT, kind="ExternalInput")
    d_out = nc.dram_tensor("o", [128, OUT_COLS], F32, kind="ExternalOutput")

    Copy = mybir.ActivationFunctionType.Copy

    with tile.TileContext(nc) as tc:
        with (
            tc.tile_pool(name="big", bufs=1) as big,
            tc.tile_pool(name="wpool", bufs=3) as wpool,
            tc.tile_pool(name="stage", bufs=3) as stage,
            tc.tile_pool(name="wps", bufs=2, space="PSUM") as wps_pool,
            tc.tile_pool(name="obank", bufs=4, space="PSUM") as obank,
        ):
            clms_sb = big.tile([LSIZE, T], WDT)
            nc.sync.dma_start(clms_sb[:], d_clms[:])
            NQ = 4
            stq = ST_COLS // NQ
            st_sbs = [big.tile([LSIZE, stq], WDT, name=f"st_sb{q}") for q in range(NQ)]
            for q in range(NQ):
                nc.sync.dma_start(st_sbs[q][:], d_st[:, stq * q : stq * (q + 1)])
            a1q = A1_COLS // 2
            a1_sbs = [big.tile([128, a1q], WDT, name=f"a1_sb{h}") for h in range(2)]
            for h in range(2):
                nc.sync.dma_start(a1_sbs[h][:], d_a1[:, a1q * h : a1q * (h + 1)])

            w_tiles = [None] * GROUPS

            def stage1(g):
                wt = wpool.tile([128, GROUP_CHUNKS * T], WDT, name=f"w_{g}",
                                tag="w")
                for j in range(GROUP_CHUNKS):
                    c = GROUP_CHUNKS * g + j
                    q, qc = divmod(c, CHUNKS // 4)
                    wps = wps_pool.tile([128, T], F32, name=f"wps_{c}", tag="wps")
                    nc.tensor.matmul(
                        wps[:],
                        st_sbs[q][:, T * qc : T * (qc + 1)],
                        clms_sb[:],
                        start=True, stop=True,
                    )
                    dst = wt[:, T * j : T * (j + 1)]
                    if j % 2 == 0:
                        nc.vector.tensor_copy(dst, wps[:])
                    else:
                        nc.scalar.activation(dst, wps[:], Copy)
                w_tiles[g] = wt

            def stage2(g, o_tile):
                for b in range(BANDS):
                    bank = obank.tile([128, 512], F32, name=f"bank_{g}_{b}",
                                      tag="obank")
                    p = SLOT_P * b
                    h, hg = divmod(g, GROUPS // 2)
                    nc.tensor.matmul(
                        bank[:],
                        a1_sbs[h][p : p + SLOT_P, T * hg : T * (hg + 1)],
                        w_tiles[g][p : p + SLOT_P, :],
                        start=True, stop=True,
                        tile_position=(p, 0),
                    )
                    dst = o_tile[:, 512 * b : 512 * (b + 1)]
                    if (b + (0 if use_f32r else g)) % 4 < 2:
                        nc.vector.tensor_copy(dst, bank[:])
                    else:
                        nc.scalar.activation(dst, bank[:], Copy)

            stage1(0)
            for g in range(GROUPS):
                if g + 1 < GROUPS:
                    stage1(g + 1)
                o_tile = stage.tile([128, 2048], F32, name=f"o_{g}", tag="o")
                stage2(g, o_tile)
                nc.sync.dma_start(d_out[:, 2048 * g : 2048 * (g + 1)], o_tile[:])
    nc.compile()
    return nc


def _install_profile_hook():
    try:
        import antenv
        from concourse import bass_utils
        if "antenv.axon_hooks" not in sys.modules:
            mod = types.ModuleType("antenv.axon_hooks")
            mod._hook = None
            mod.set_axon_ntff_profile_hook = lambda h: setattr(mod, "_hook", h)
            mod.get_axon_ntff_profile_hook = lambda: mod._hook
            sys.modules["antenv.axon_hooks"] = mod
            antenv.axon_hooks = mod
        from trn_agent_boot.trn_boot import _ntff_profile_via_ctypes
        sys.modules["antenv.axon_hooks"].set_axon_ntff_profile_hook(
            _ntff_profile_via_ctypes("/opt/axon/libaxon_pjrt.so"))
        bass_utils.upload_artifacts = lambda tmpdir: f"local:{tmpdir}"
    except Exception as e:
        print(f"kernel: profile hook unavailable ({e})", file=sys.stderr)


def _build_core_inputs(clms, C, core):
    s_t = np.zeros((LSIZE, ST_COLS), np.float32)
    a1s = np.zeros((128, A1_COLS), np.float32)
    base = CELLS_PER_CORE * core
    for c_loc in range(CELLS_PER_CORE):
        ci = base + c_loc
        if ci >= N_CELLS:
            break
        pi, kb = CELL_TABLE[ci]
        l1, l2, lo, hi, rows, po = PAIRS[pi]
        d1, d2 = 2 * l1 + 1, 2 * l2 + 1
        g, b = divmod(c_loc, BANDS)
        a1s[SLOT_P * b : SLOT_P * b + d1, T * g : T * (g + 1)] = \
            clms[l1 * l1 : l1 * l1 + d1, :]
        kmax = (hi + 1) ** 2
        for j in range(GROUP_CHUNKS):
            k = kb + j
            if k >= kmax:
                break
            col = T * (GROUP_CHUNKS * g + j) + SLOT_P * b
            blk = C[k, l1 * l1 : l1 * l1 + d1, l2 * l2 : l2 * l2 + d2]
            s_t[l2 * l2 : l2 * l2 + d2, col : col + d1] = blk.T
    return s_t, a1s


def kernel(clms, C):
    global _NC, LAST_EXEC_TIME_NS
    from concourse.bass_utils import run_bass_kernel_spmd

    trace = os.environ.get("BASS_TRACE", "0") == "1"
    use_f32r = os.environ.get("KERNEL_F32R", "0") == "1"
    if trace:
        _install_profile_hook()

    clms = np.ascontiguousarray(np.asarray(clms, dtype=np.float32))
    C = np.ascontiguousarray(np.asarray(C, dtype=np.float32))

    if _NC is None:
        _NC = _build_nc(use_f32r)

    in_maps = []
    for core in range(NCORES):
        s_t, a1s = _build_core_inputs(clms, C, core)
        in_maps.append({"clms": clms, "s_t": s_t, "a1s": a1s})

    res = run_bass_kernel_spmd(_NC, in_maps, list(range(NCORES)), trace=trace)
    LAST_EXEC_TIME_NS = res.exec_time_ns

    # ---------------- host reassembly ----------------
    G = np.empty((TOTAL_ROWS, T, T), np.float32)
    for core in range(NCORES):
        o = res.results[core]["o"]          # [128, OUT_COLS]
        base = CELLS_PER_CORE * core
        for c_loc in range(CELLS_PER_CORE):
            ci = base + c_loc
            if ci >= N_CELLS:
                break
            pi, kb = CELL_TABLE[ci]
            l1, l2, lo, hi, rows, po = PAIRS[pi]
            kmax = (hi + 1) ** 2
            g, b = divmod(c_loc, BANDS)
            for j in range(GROUP_CHUNKS):
                k = kb + j
                if k >= kmax:
                    break
                cb = 2048 * g + 512 * b + 128 * j
                G[po + (k - lo * lo)] = o[:, cb : cb + T]

    # mirror lower pairs (l1 > l2) from upper: OUT = sign * OUT_upper^T
    ls = np.arange(LSIZE)
    l_of_k = np.floor(np.sqrt(ls)).astype(np.int64)
    for pi, (l1, l2, lo, hi, rows, po) in enumerate(PAIRS):
        if l1 <= l2:
            continue
        up = PAIRS[8 * l2 + l1]
        po_u = up[5]
        ks = np.arange(lo * lo, (hi + 1) ** 2)
        sign = ((-1.0) ** (l1 + l2 - l_of_k[ks])).astype(np.float32)
        G[po : po + rows] = sign[:, None, None] * \
            G[po_u : po_u + rows].transpose(0, 2, 1)

    Gf = G.reshape(TOTAL_ROWS, T * T)
    out = []
    for l in range(L + 1):
        blocks = []
        for (l1, l2, lo, hi, rows, po) in PAIRS:
            if lo <= l <= hi:
                r0 = po + (l * l - lo * lo)
                blocks.append(Gf[r0 : r0 + 2 * l + 1, :])
        out.append(np.concatenate(blocks, axis=1))
    return tuple(out)


# revision 6
# speedup vs baseline: 1.1087x; 1.1087x over previous
"""Trainium2 Bass kernel for nn_CgTransform (L=7, T=128, 8 NeuronCores).

Math: for each (l1,l2) block pair and each kept output row k=(l,m):
    OUT_k[t1,t2] = sum_{i,j} C[k, l1^2+i, l2^2+j] * clms[l1^2+i, t1] * clms[l2^2+j, t2]
                 = A1^T (C_k A2).

Symmetry (verified numerically): CG coefficient symmetry gives
    OUT_{(l2,l1),k} = (-1)^{l1+l2-l} * OUT_{(l1,l2),k}^T,
so the device computes only the 36 pairs with l1<=l2 (1378 of 2416 output
rows); the host mirrors the rest by transposing [T,T] blocks.

Device restructuring (per output row k = one "slot"):
  stage 1:  W_k[i, t2] = sum_j C_k[i,j] * clms[l2^2+j, t2]
            -> dense matmul W_chunk = S_chunk^T.T @ clms per 128-partition
               chunk; 4 slots per chunk at 32-partition bands.
  stage 2:  OUT_k[t1,t2] = sum_i A1pad[i,t1] * W_k[i,t2]
            -> slots are grouped into CELLS of 4 consecutive k of the SAME
               pair spread over the 4 chunks of a group, so one N=512 matmul
               (lhsT = padded A1 at band 32b, rhs = the group's W band rows)
               computes 4 slots. tile_position=(32b, 0); each cell gets its
               own PSUM bank (HW: different tile_positions must not share a
               bank).

Precision modes (env KERNEL_MODE):
  f16x2 (default): every operand is split x = hi + lo into two fp16 halves
        and each matmul is done as 3 fp16 passes (hi*hi + hi*lo + lo*hi)
        accumulating in fp32 PSUM. fp16 keeps 11 mantissa bits, so the
        dropped lo*lo term and split roundings are ~2^-22 relative: the
        result matches fp32 matmuls (~2e-7) at bf16 speed (1 cycle/column,
        fast weight load) instead of fp32's 4 cycles/column.
  f32:  plain fp32 matmuls (4 cyc/col).
  f32r: float32r (TF32-like, ~4e-4 rel err) - fastest, reduced precision.

Sharding: 384 padded cells split 48 per core (identical SPMD program,
per-core S/A1 data). Output layout per core: [t1, group, band, j, t2]; host
transposes to [k, t1, t2] and reassembles the 8-tuple.
"""
import os
import sys
import types

if "/opt/trn_rl_repo" not in sys.path:
    sys.path.insert(0, "/opt/trn_rl_repo")

import numpy as np

L = 7
LSIZE = (L + 1) ** 2          # 64
T = 128
NCORES = 8
SLOT_P = 32                   # partitions per slot band
BANDS = 4                     # bands (cells) per group
GROUP_CHUNKS = 4              # chunks per group == slots per cell

# ------------------------------------------------------------ pair tables
PAIRS = []                    # full table: (l1, l2, lo, hi, rows, row_offset)
_off = 0
for _l1 in range(L + 1):
    for _l2 in range(L + 1):
        _lo, _hi = abs(_l1 - _l2), min(_l1 + _l2, L)
        _rows = (_hi + 1) ** 2 - _lo * _lo
        PAIRS.append((_l1, _l2, _lo, _hi, _rows, _off))
        _off += _rows
TOTAL_ROWS = _off             # 2416

UP_PAIRS = [i for i, p in enumerate(PAIRS) if p[0] <= p[1]]   # 36 pair idxs

# global cell list: (full_pair_idx, k_base); one pair per cell, 4 k's
CELL_TABLE = []
for _pi in UP_PAIRS:
    _l1, _l2, _lo, _hi, _rows, _po = PAIRS[_pi]
    for _kb in range(_lo * _lo, (_hi + 1) ** 2, GROUP_CHUNKS):
        CELL_TABLE.append((_pi, _kb))
N_CELLS = len(CELL_TABLE)     # 356

CELLS_PER_CORE = -(-(-(-N_CELLS // NCORES)) // BANDS) * BANDS  # 48
GROUPS = CELLS_PER_CORE // BANDS       # 12
CHUNKS = GROUPS * GROUP_CHUNKS         # 48
SLOTS = CELLS_PER_CORE * GROUP_CHUNKS  # 192 per core
ST_COLS = CHUNKS * T                   # 6144
A1_COLS = GROUPS * T                   # 1536
OUT_COLS = SLOTS * T                   # 24576

_NC = None
_NC_MODE = None
LAST_EXEC_TIME_NS = None


def _build_nc(mode):
    import concourse.bacc as bacc
    import concourse.mybir as mybir
    import concourse.tile as tile

    F32 = mybir.dt.float32
    F16 = mybir.dt.float16
    split = mode == "f16x2"
    WDT = {"f32": F32, "f32r": mybir.dt.float32r, "f16x2": F16}[mode]

    nc = bacc.Bacc("TRN2", target_bir_lowering=False, debug=False, num_devices=1)
    nhl = 2 if split else 1
    d_clms = [nc.dram_tensor(f"clms{h}", [LSIZE, T], WDT, kind="ExternalInput")
              for h in range(nhl)]
    d_st = [nc.dram_tensor(f"s_t{h}", [LSIZE, ST_COLS], WDT, kind="ExternalInput")
            for h in range(nhl)]
    d_a1 = [nc.dram_tensor(f"a1s{h}", [128, A1_COLS], WDT, kind="ExternalInput")
            for h in range(nhl)]
    d_out = nc.dram_tensor("o", [128, OUT_COLS], F32, kind="ExternalOutput")

    Copy = mybir.ActivationFunctionType.Copy

    with tile.TileContext(nc) as tc:
        with (
            tc.tile_pool(name="big", bufs=1) as big,
            tc.tile_pool(name="wpool", bufs=3) as wpool,
            tc.tile_pool(name="stage", bufs=3) as stage,
            tc.tile_pool(name="wps", bufs=2, space="PSUM") as wps_pool,
            tc.tile_pool(name="obank", bufs=4, space="PSUM") as obank,
        ):
            clms_sb = [big.tile([LSIZE, T], WDT, name=f"clms_sb{h}")
                       for h in range(nhl)]
            for h in range(nhl):
                nc.sync.dma_start(clms_sb[h][:], d_clms[h][:])
            NQ = 4
            stq = ST_COLS // NQ
            st_sbs = [[big.tile([LSIZE, stq], WDT, name=f"st_sb{h}_{q}")
                       for q in range(NQ)] for h in range(nhl)]
            for q in range(NQ):
                for h in range(nhl):
                    nc.sync.dma_start(st_sbs[h][q][:],
                                      d_st[h][:, stq * q : stq * (q + 1)])
            a1q = A1_COLS // 2
            a1_sbs = [[big.tile([128, a1q], WDT, name=f"a1_sb{h}_{p}")
                       for p in range(2)] for h in range(nhl)]
            for p in range(2):
                for h in range(nhl):
                    nc.sync.dma_start(a1_sbs[h][p][:],
                                      d_a1[h][:, a1q * p : a1q * (p + 1)])

            w_tiles = [None] * GROUPS     # per group: [wt_h] or [wt_h, wt_l]

            def stage1(g):
                wts = [wpool.tile([128, GROUP_CHUNKS * T], WDT,
                                  name=f"w_{g}_{h}", tag=f"w{h}")
                       for h in range(nhl)]
                for j in range(GROUP_CHUNKS):
                    c = GROUP_CHUNKS * g + j
                    q, qc = divmod(c, CHUNKS // NQ)
                    wps = wps_pool.tile([128, T], F32, name=f"wps_{c}", tag="wps")
                    if split:
                        passes = [(0, 0), (0, 1), (1, 0)]
                        for n, (hs, hc) in enumerate(passes):
                            nc.tensor.matmul(
                                wps[:],
                                st_sbs[hs][q][:, T * qc : T * (qc + 1)],
                                clms_sb[hc][:],
                                start=(n == 0), stop=(n == len(passes) - 1),
                            )
                    else:
                        nc.tensor.matmul(
                            wps[:],
                            st_sbs[0][q][:, T * qc : T * (qc + 1)],
                            clms_sb[0][:],
                            start=True, stop=True,
                        )
                    dst_h = wts[0][:, T * j : T * (j + 1)]
                    if split:
                        # Wh = f16(W); Wl = f16(W - Wh)
                        nc.scalar.activation(dst_h, wps[:], Copy)
                        nc.vector.tensor_sub(wts[1][:, T * j : T * (j + 1)],
                                             wps[:], dst_h)
                    elif j % 2 == 0:
                        nc.vector.tensor_copy(dst_h, wps[:])
                    else:
                        nc.scalar.activation(dst_h, wps[:], Copy)
                w_tiles[g] = wts

            def stage2(g, o_tile):
                h2, hg = divmod(g, GROUPS // 2)
                for b in range(BANDS):
                    bank = obank.tile([128, 512], F32, name=f"bank_{g}_{b}",
                                      tag="obank")
                    p = SLOT_P * b
                    a1ap = [a1_sbs[h][h2][p : p + SLOT_P, T * hg : T * (hg + 1)]
                            for h in range(nhl)]
                    wap = [w_tiles[g][h][p : p + SLOT_P, :] for h in range(nhl)]
                    if split:
                        passes = [(0, 0), (0, 1), (1, 0)]
                        for n, (ha, hw) in enumerate(passes):
                            nc.tensor.matmul(
                                bank[:], a1ap[ha], wap[hw],
                                start=(n == 0), stop=(n == len(passes) - 1),
                                tile_position=(p, 0),
                            )
                    else:
                        nc.tensor.matmul(
                            bank[:], a1ap[0], wap[0],
                            start=True, stop=True,
                            tile_position=(p, 0),
                        )
                    dst = o_tile[:, 512 * b : 512 * (b + 1)]
                    if b % 2 == 0:
                        nc.vector.tensor_copy(dst, bank[:])
                    else:
                        nc.scalar.activation(dst, bank[:], Copy)

            stage1(0)
            for g in range(GROUPS):
                if g + 1 < GROUPS:
                    stage1(g + 1)
                o_tile = stage.tile([128, 2048], F32, name=f"o_{g}", tag="o")
                stage2(g, o_tile)
                nc.sync.dma_start(d_out[:, 2048 * g : 2048 * (g + 1)], o_tile[:])
    nc.compile()
    return nc


def _install_profile_hook():
    try:
        import antenv
        from concourse import bass_utils
        if "antenv.axon_hooks" not in sys.modules:
            mod = types.ModuleType("antenv.axon_hooks")
            mod._hook = None
            mod.set_axon_ntff_profile_hook = lambda h: setattr(mod, "_hook", h)
            mod.get_axon_ntff_profile_hook = lambda: mod._hook
            sys.modules["antenv.axon_hooks"] = mod
            antenv.axon_hooks = mod
        from trn_agent_boot.trn_boot import _ntff_profile_via_ctypes
        sys.modules["antenv.axon_hooks"].set_axon_ntff_profile_hook(
            _ntff_profile_via_ctypes("/opt/axon/libaxon_pjrt.so"))
        bass_utils.upload_artifacts = lambda tmpdir: f"local:{tmpdir}"
    except Exception as e:
        print(f"kernel: profile hook unavailable ({e})", file=sys.stderr)


def _build_core_inputs(clms, C, core):
    """Dense fp32 s_t [64, ST_COLS] and a1s [128, A1_COLS] for one core."""
    s_t = np.zeros((LSIZE, ST_COLS), np.float32)
    a1s = np.zeros((128, A1_COLS), np.float32)
    base = CELLS_PER_CORE * core
    for c_loc in range(CELLS_PER_CORE):
        ci = base + c_loc
        if ci >= N_CELLS:
            break
        pi, kb = CELL_TABLE[ci]
        l1, l2, lo, hi, rows, po = PAIRS[pi]
        d1, d2 = 2 * l1 + 1, 2 * l2 + 1
        g, b = divmod(c_loc, BANDS)
        a1s[SLOT_P * b : SLOT_P * b + d1, T * g : T * (g + 1)] = \
            clms[l1 * l1 : l1 * l1 + d1, :]
        kmax = (hi + 1) ** 2
        for j in range(GROUP_CHUNKS):
            k = kb + j
            if k >= kmax:
                break
            col = T * (GROUP_CHUNKS * g + j) + SLOT_P * b
            blk = C[k, l1 * l1 : l1 * l1 + d1, l2 * l2 : l2 * l2 + d2]
            s_t[l2 * l2 : l2 * l2 + d2, col : col + d1] = blk.T
    return s_t, a1s


def _split16(x):
    hi = x.astype(np.float16)
    lo = (x - hi.astype(np.float32)).astype(np.float16)
    return hi, lo


def kernel(clms, C):
    global _NC, _NC_MODE, LAST_EXEC_TIME_NS
    from concourse.bass_utils import run_bass_kernel_spmd

    trace = os.environ.get("BASS_TRACE", "0") == "1"
    mode = os.environ.get("KERNEL_MODE", "f16x2")
    if trace:
        _install_profile_hook()

    clms = np.ascontiguousarray(np.asarray(clms, dtype=np.float32))
    C = np.ascontiguousarray(np.asarray(C, dtype=np.float32))

    if _NC is None or _NC_MODE != mode:
        _NC = _build_nc(mode)
        _NC_MODE = mode

    in_maps = []
    for core in range(NCORES):
        s_t, a1s = _build_core_inputs(clms, C, core)
        if mode == "f16x2":
            sh, sl = _split16(s_t)
            ah, al = _split16(a1s)
            ch, cl = _split16(clms)
            in_maps.append({"clms0": ch, "clms1": cl, "s_t0": sh, "s_t1": sl,
                            "a1s0": ah, "a1s1": al})
        else:
            in_maps.append({"clms0": clms, "s_t0": s_t, "a1s0": a1s})

    res = run_bass_kernel_spmd(_NC, in_maps, list(range(NCORES)), trace=trace)
    LAST_EXEC_TIME_NS = res.exec_time_ns

    # ---------------- host reassembly ----------------
    G = np.empty((TOTAL_ROWS, T, T), np.float32)
    for core in range(NCORES):
        o = res.results[core]["o"]          # [128, OUT_COLS]
        base = CELLS_PER_CORE * core
        for c_loc in range(CELLS_PER_CORE):
            ci = base + c_loc
            if ci >= N_CELLS:
                break
            pi, kb = CELL_TABLE[ci]
            l1, l2, lo, hi, rows, po = PAIRS[pi]
            kmax = (hi + 1) ** 2
            g, b = divmod(c_loc, BANDS)
            for j in range(GROUP_CHUNKS):
                k = kb + j
                if k >= kmax:
                    break
                cb = 2048 * g + 512 * b + 128 * j
                G[po + (k - lo * lo)] = o[:, cb : cb + T]

    # mirror lower pairs (l1 > l2) from upper: OUT = sign * OUT_upper^T
    ls = np.arange(LSIZE)
    l_of_k = np.floor(np.sqrt(ls)).astype(np.int64)
    for pi, (l1, l2, lo, hi, rows, po) in enumerate(PAIRS):
        if l1 <= l2:
            continue
        po_u = PAIRS[8 * l2 + l1][5]
        ks = np.arange(lo * lo, (hi + 1) ** 2)
        sign = ((-1.0) ** (l1 + l2 - l_of_k[ks])).astype(np.float32)
        G[po : po + rows] = sign[:, None, None] * \
            G[po_u : po_u + rows].transpose(0, 2, 1)

    Gf = G.reshape(TOTAL_ROWS, T * T)
    out = []
    for l in range(L + 1):
        blocks = []
        for (l1, l2, lo, hi, rows, po) in PAIRS:
            if lo <= l <= hi:
                r0 = po + (l * l - lo * lo)
                blocks.append(Gf[r0 : r0 + 2 * l + 1, :])
        out.append(np.concatenate(blocks, axis=1))
    return tuple(out)


# revision 7
# speedup vs baseline: 1.3249x; 1.1950x over previous
"""Trainium2 Bass kernel for nn_CgTransform (L=7, T=128, 8 NeuronCores).

Math: for each (l1,l2) block pair and each kept output row k=(l,m):
    OUT_k[t1,t2] = sum_{i,j} C[k, l1^2+i, l2^2+j] * clms[l1^2+i, t1] * clms[l2^2+j, t2]
                 = A1^T (C_k A2).

Symmetry (verified numerically): CG coefficient symmetry gives
    OUT_{(l2,l1),k} = (-1)^{l1+l2-l} * OUT_{(l1,l2),k}^T,
so the device computes only the 36 pairs with l1<=l2 (1378 of 2416 output
rows); the host mirrors the rest by transposing [T,T] blocks.

Device restructuring (per output row k = one "slot"):
  stage 1:  W_k[i, t2] = sum_j C_k[i,j] * clms[l2^2+j, t2]
            -> dense matmul W_chunk = S_chunk^T.T @ clms per 128-partition
               chunk; 4 slots per chunk at 32-partition bands.
  stage 2:  OUT_k[t1,t2] = sum_i A1pad[i,t1] * W_k[i,t2]
            -> slots are grouped into CELLS of 4 consecutive k of the SAME
               pair spread over the 4 chunks of a group, so one N=512 matmul
               (lhsT = padded A1 at band 32b, rhs = the group's W band rows)
               computes 4 slots. tile_position=(32b, 0); each cell gets its
               own PSUM bank (HW: different tile_positions must not share a
               bank).

Precision modes (env KERNEL_MODE):
  f16x2 (default): operands split x = hi + lo into fp16 halves; matmuls run
        as fp16 passes accumulating in fp32 PSUM (dropped lo*lo terms are
        ~2^-22 relative), matching fp32 accuracy (~3e-7) at 1 cycle/column.
        Stage 1 merges its hi*lo+lo*hi passes into one 128-contraction
        matmul via operand stacking (2 matmuls total). Stage 2 runs 3
        passes, emitted pass-major across bands so LDWEIGHTS of one band
        overlaps the matmul of another (different PE row groups).
  f32:  plain fp32 matmuls (4 cyc/col).
  f32r: float32r (TF32-like, ~4e-4 rel err) - fast but reduced precision.

Sharding: 384 padded cells split 48 per core (identical SPMD program,
per-core S/A1 data). Output layout per core: [t1, group, band, j, t2]; host
transposes to [k, t1, t2] and reassembles the 8-tuple.
"""
import os
import sys
import types

if "/opt/trn_rl_repo" not in sys.path:
    sys.path.insert(0, "/opt/trn_rl_repo")

import numpy as np

L = 7
LSIZE = (L + 1) ** 2          # 64
T = 128
NCORES = 8
SLOT_P = 32                   # partitions per slot band
BANDS = 4                     # bands (cells) per group
GROUP_CHUNKS = 4              # chunks per group == slots per cell

# ------------------------------------------------------------ pair tables
PAIRS = []                    # full table: (l1, l2, lo, hi, rows, row_offset)
_off = 0
for _l1 in range(L + 1):
    for _l2 in range(L + 1):
        _lo, _hi = abs(_l1 - _l2), min(_l1 + _l2, L)
        _rows = (_hi + 1) ** 2 - _lo * _lo
        PAIRS.append((_l1, _l2, _lo, _hi, _rows, _off))
        _off += _rows
TOTAL_ROWS = _off             # 2416

UP_PAIRS = [i for i, p in enumerate(PAIRS) if p[0] <= p[1]]   # 36 pair idxs

# global cell list: (full_pair_idx, k_base); one pair per cell, 4 k's
CELL_TABLE = []
for _pi in UP_PAIRS:
    _l1, _l2, _lo, _hi, _rows, _po = PAIRS[_pi]
    for _kb in range(_lo * _lo, (_hi + 1) ** 2, GROUP_CHUNKS):
        CELL_TABLE.append((_pi, _kb))
N_CELLS = len(CELL_TABLE)     # 356

CELLS_PER_CORE = -(-(-(-N_CELLS // NCORES)) // BANDS) * BANDS  # 48
GROUPS = CELLS_PER_CORE // BANDS       # 12
CHUNKS = GROUPS * GROUP_CHUNKS         # 48
SLOTS = CELLS_PER_CORE * GROUP_CHUNKS  # 192 per core
ST_COLS = CHUNKS * T                   # 6144
A1_COLS = GROUPS * T                   # 1536
OUT_COLS = SLOTS * T                   # 24576
DMA_GROUPS = 2                         # output groups batched per dma_start

_NC = None
_NC_MODE = None
LAST_EXEC_TIME_NS = None


def _build_nc(mode):
    import concourse.bacc as bacc
    import concourse.mybir as mybir
    import concourse.tile as tile

    F32 = mybir.dt.float32
    split = mode == "f16x2"
    WDT = {"f32": F32, "f32r": mybir.dt.float32r,
           "f16x2": mybir.dt.float16}[mode]
    ST_P = 128 if split else 64        # stb rows: [Sh; Sl] stacked when split
    A1W = (2 if split else 1) * A1_COLS

    nc = bacc.Bacc("TRN2", target_bir_lowering=False, debug=False, num_devices=1)
    d_stb = nc.dram_tensor("stb", [ST_P, ST_COLS], WDT, kind="ExternalInput")
    d_cla = nc.dram_tensor("cla", [LSIZE, T], WDT, kind="ExternalInput")
    d_clb = (nc.dram_tensor("clb", [128, T], WDT, kind="ExternalInput")
             if split else None)
    d_a1 = nc.dram_tensor("a1b", [128, A1W], WDT, kind="ExternalInput")
    d_out = nc.dram_tensor("o", [128, OUT_COLS], F32, kind="ExternalOutput")

    Copy = mybir.ActivationFunctionType.Copy

    with tile.TileContext(nc) as tc:
        with (
            tc.tile_pool(name="big", bufs=1) as big,
            tc.tile_pool(name="wpool", bufs=3) as wpool,
            tc.tile_pool(name="stage", bufs=2) as stage,
            tc.tile_pool(name="wps", bufs=3, space="PSUM") as wps_pool,
            tc.tile_pool(name="obank", bufs=4, space="PSUM") as obank,
        ):
            cla_sb = big.tile([LSIZE, T], WDT, name="cla_sb")
            nc.sync.dma_start(cla_sb[:], d_cla[:])
            if split:
                clb_sb = big.tile([128, T], WDT, name="clb_sb")
                nc.sync.dma_start(clb_sb[:], d_clb[:])
            NQ = 4
            stq = ST_COLS // NQ
            st_sbs = [big.tile([ST_P, stq], WDT, name=f"st_sb{q}")
                      for q in range(NQ)]
            for q in range(NQ):
                nc.sync.dma_start(st_sbs[q][:], d_stb[:, stq * q : stq * (q + 1)])
            a1_sb = big.tile([128, A1W], WDT, name="a1_sb")
            nc.sync.dma_start(a1_sb[:], d_a1[:])

            w_tiles = [None] * GROUPS     # per group: [wt_h] or [wt_h, wt_l]

            def s1_tiles(g):
                w_tiles[g] = [wpool.tile([128, GROUP_CHUNKS * T], WDT,
                                         name=f"w_{g}_{h}", tag=f"w{h}")
                              for h in range(2 if split else 1)]
                return [None] * GROUP_CHUNKS

            def s1_mm(g, j, wpss):
                c = GROUP_CHUNKS * g + j
                q, qc = divmod(c, CHUNKS // NQ)
                wps = wps_pool.tile([128, T], mybir.dt.float32,
                                    name=f"wps_{c}", tag="wps")
                if split:
                    nc.tensor.matmul(wps[:],
                                     st_sbs[q][0:64, T * qc : T * (qc + 1)],
                                     cla_sb[:], start=True, stop=False)
                    nc.tensor.matmul(wps[:],
                                     st_sbs[q][:, T * qc : T * (qc + 1)],
                                     clb_sb[:], start=False, stop=True)
                else:
                    nc.tensor.matmul(wps[:],
                                     st_sbs[q][:, T * qc : T * (qc + 1)],
                                     cla_sb[:], start=True, stop=True)
                wpss[j] = wps

            def s1_copy(g, j, wpss):
                wps = wpss[j]
                dst_h = w_tiles[g][0][:, T * j : T * (j + 1)]
                if split:
                    nc.scalar.activation(dst_h, wps[:], Copy)
                    nc.vector.tensor_sub(w_tiles[g][1][:, T * j : T * (j + 1)],
                                         wps[:], dst_h)
                elif j % 2 == 0:
                    nc.vector.tensor_copy(dst_h, wps[:])
                else:
                    nc.scalar.activation(dst_h, wps[:], Copy)

            def s2_banks(g):
                return [obank.tile([128, 512], mybir.dt.float32,
                                   name=f"bank_{g}_{b}", tag="obank")
                        for b in range(BANDS)]

            def s2_pass(g, banks, n, npass):
                # pass-major emission: LDWEIGHTS of band b overlaps the
                # in-flight matmul of band b-1 (different PE row groups)
                if split:
                    ha, hw = [(0, 0), (0, 1), (1, 0)][n]
                else:
                    ha, hw = 0, 0
                for b in range(BANDS):
                    p = SLOT_P * b
                    a1ap = a1_sb[p : p + SLOT_P,
                                 A1_COLS * ha + T * g : A1_COLS * ha + T * (g + 1)]
                    nc.tensor.matmul(
                        banks[b][:], a1ap, w_tiles[g][hw][p : p + SLOT_P, :],
                        start=(n == 0), stop=(n == npass - 1),
                        tile_position=(p, 0),
                    )

            def s2_copy(g, banks, o_tile):
                off = 2048 * (g % DMA_GROUPS)
                for b in range(BANDS):
                    dst = o_tile[:, off + 512 * b : off + 512 * (b + 1)]
                    if b % 2 == 0:
                        nc.vector.tensor_copy(dst, banks[b][:])
                    else:
                        nc.scalar.activation(dst, banks[b][:], Copy)

            npass = 3 if split else 1
            # prologue: stage-1 for group 0
            wpss = s1_tiles(0)
            for j in range(GROUP_CHUNKS):
                s1_mm(0, j, wpss)
                s1_copy(0, j, wpss)
            o_tile = None
            for g in range(GROUPS):
                if g % DMA_GROUPS == 0:
                    o_tile = stage.tile([128, 2048 * DMA_GROUPS],
                                        mybir.dt.float32, name=f"o_{g}", tag="o")
                banks = s2_banks(g)
                if g + 1 < GROUPS:
                    # interleave next group's stage-1 matmuls between passes
                    wpss = s1_tiles(g + 1)
                    for n in range(npass):
                        s1_mm(g + 1, n, wpss)
                        s1_copy(g + 1, n, wpss)
                        s2_pass(g, banks, n, npass)
                    for j in range(npass, GROUP_CHUNKS):
                        s1_mm(g + 1, j, wpss)
                        s1_copy(g + 1, j, wpss)
                else:
                    for n in range(npass):
                        s2_pass(g, banks, n, npass)
                s2_copy(g, banks, o_tile)
                if (g + 1) % DMA_GROUPS == 0:
                    g0 = g + 1 - DMA_GROUPS
                    nc.sync.dma_start(
                        d_out[:, 2048 * g0 : 2048 * (g + 1)], o_tile[:])
    nc.compile()
    return nc


def _install_profile_hook():
    try:
        import antenv
        from concourse import bass_utils
        if "antenv.axon_hooks" not in sys.modules:
            mod = types.ModuleType("antenv.axon_hooks")
            mod._hook = None
            mod.set_axon_ntff_profile_hook = lambda h: setattr(mod, "_hook", h)
            mod.get_axon_ntff_profile_hook = lambda: mod._hook
            sys.modules["antenv.axon_hooks"] = mod
            antenv.axon_hooks = mod
        from trn_agent_boot.trn_boot import _ntff_profile_via_ctypes
        sys.modules["antenv.axon_hooks"].set_axon_ntff_profile_hook(
            _ntff_profile_via_ctypes("/opt/axon/libaxon_pjrt.so"))
        bass_utils.upload_artifacts = lambda tmpdir: f"local:{tmpdir}"
    except Exception as e:
        print(f"kernel: profile hook unavailable ({e})", file=sys.stderr)


def _build_core_inputs(clms, C, core):
    """Dense fp32 s_t [64, ST_COLS] and a1s [128, A1_COLS] for one core."""
    s_t = np.zeros((LSIZE, ST_COLS), np.float32)
    a1s = np.zeros((128, A1_COLS), np.float32)
    base = CELLS_PER_CORE * core
    for c_loc in range(CELLS_PER_CORE):
        ci = base + c_loc
        if ci >= N_CELLS:
            break
        pi, kb = CELL_TABLE[ci]
        l1, l2, lo, hi, rows, po = PAIRS[pi]
        d1, d2 = 2 * l1 + 1, 2 * l2 + 1
        g, b = divmod(c_loc, BANDS)
        a1s[SLOT_P * b : SLOT_P * b + d1, T * g : T * (g + 1)] = \
            clms[l1 * l1 : l1 * l1 + d1, :]
        kmax = (hi + 1) ** 2
        for j in range(GROUP_CHUNKS):
            k = kb + j
            if k >= kmax:
                break
            col = T * (GROUP_CHUNKS * g + j) + SLOT_P * b
            blk = C[k, l1 * l1 : l1 * l1 + d1, l2 * l2 : l2 * l2 + d2]
            s_t[l2 * l2 : l2 * l2 + d2, col : col + d1] = blk.T
    return s_t, a1s


def _split16(x):
    hi = x.astype(np.float16)
    lo = (x - hi.astype(np.float32)).astype(np.float16)
    return hi, lo


def kernel(clms, C):
    global _NC, _NC_MODE, LAST_EXEC_TIME_NS
    from concourse.bass_utils import run_bass_kernel_spmd

    trace = os.environ.get("BASS_TRACE", "0") == "1"
    mode = os.environ.get("KERNEL_MODE", "f16x2")
    if trace:
        _install_profile_hook()

    clms = np.ascontiguousarray(np.asarray(clms, dtype=np.float32))
    C = np.ascontiguousarray(np.asarray(C, dtype=np.float32))

    if _NC is None or _NC_MODE != mode:
        _NC = _build_nc(mode)
        _NC_MODE = mode

    in_maps = []
    for core in range(NCORES):
        s_t, a1s = _build_core_inputs(clms, C, core)
        if mode == "f16x2":
            sh, sl = _split16(s_t)
            ah, al = _split16(a1s)
            ch, cl = _split16(clms)
            in_maps.append({
                "stb": np.concatenate([sh, sl], axis=0),
                "cla": ch,
                "clb": np.concatenate([cl, ch], axis=0),
                "a1b": np.concatenate([ah, al], axis=1),
            })
        else:
            in_maps.append({"stb": s_t, "cla": clms, "a1b": a1s})

    res = run_bass_kernel_spmd(_NC, in_maps, list(range(NCORES)), trace=trace)
    LAST_EXEC_TIME_NS = res.exec_time_ns

    # ---------------- host reassembly ----------------
    G = np.empty((TOTAL_ROWS, T, T), np.float32)
    for core in range(NCORES):
        o = res.results[core]["o"]          # [128, OUT_COLS]
        base = CELLS_PER_CORE * core
        for c_loc in range(CELLS_PER_CORE):
            ci = base + c_loc
            if ci >= N_CELLS:
                break
            pi, kb = CELL_TABLE[ci]
            l1, l2, lo, hi, rows, po = PAIRS[pi]
            kmax = (hi + 1) ** 2
            g, b = divmod(c_loc, BANDS)
            for j in range(GROUP_CHUNKS):
                k = kb + j
                if k >= kmax:
                    break
                cb = 2048 * g + 512 * b + 128 * j
                G[po + (k - lo * lo)] = o[:, cb : cb + T]

    # mirror lower pairs (l1 > l2) from upper: OUT = sign * OUT_upper^T
    ls = np.arange(LSIZE)
    l_of_k = np.floor(np.sqrt(ls)).astype(np.int64)
    for pi, (l1, l2, lo, hi, rows, po) in enumerate(PAIRS):
        if l1 <= l2:
            continue
        po_u = PAIRS[8 * l2 + l1][5]
        ks = np.arange(lo * lo, (hi + 1) ** 2)
        sign = ((-1.0) ** (l1 + l2 - l_of_k[ks])).astype(np.float32)
        G[po : po + rows] = sign[:, None, None] * \
            G[po_u : po_u + rows].transpose(0, 2, 1)

    Gf = G.reshape(TOTAL_ROWS, T * T)
    out = []
    for l in range(L + 1):
        blocks = []
        for (l1, l2, lo, hi, rows, po) in PAIRS:
            if lo <= l <= hi:
                r0 = po + (l * l - lo * lo)
                blocks.append(Gf[r0 : r0 + 2 * l + 1, :])
        out.append(np.concatenate(blocks, axis=1))
    return tuple(out)


# revision 8
# speedup vs baseline: 1.4243x; 1.0750x over previous
"""Trainium2 Bass kernel for nn_CgTransform (L=7, T=128, 8 NeuronCores).

Math: for each (l1,l2) block pair and each kept output row k=(l,m):
    OUT_k[t1,t2] = sum_{i,j} C[k, l1^2+i, l2^2+j] * clms[l1^2+i, t1] * clms[l2^2+j, t2]
                 = A1^T (C_k A2).

Symmetry (verified numerically): CG coefficient symmetry gives
    OUT_{(l2,l1),k} = (-1)^{l1+l2-l} * OUT_{(l1,l2),k}^T,
so the device computes only the 36 pairs with l1<=l2 (1378 of 2416 output
rows); the host mirrors the rest by transposing [T,T] blocks.

Device restructuring (per output row k = one "slot"):
  stage 1:  W_k[i, t2] = sum_j C_k[i,j] * clms[l2^2+j, t2]
            -> dense matmul W_chunk = S_chunk^T.T @ clms per 128-partition
               chunk; 4 slots per chunk at 32-partition bands.
  stage 2:  OUT_k[t1,t2] = sum_i A1pad[i,t1] * W_k[i,t2]
            -> slots are grouped into CELLS of 4 consecutive k of the SAME
               pair spread over the 4 chunks of a group, so one N=512 matmul
               (lhsT = padded A1 at band 32b, rhs = the group's W band rows)
               computes 4 slots. tile_position=(32b, 0); each cell gets its
               own PSUM bank (HW: different tile_positions must not share a
               bank).

Precision modes (env KERNEL_MODE):
  f16x2 (default): operands split x = hi + lo into fp16 halves; matmuls run
        as fp16 passes accumulating in fp32 PSUM (dropped lo*lo terms are
        ~2^-22 relative), matching fp32 accuracy (~3e-7) at 1 cycle/column.
        Stage 1 merges its hi*lo+lo*hi passes into one 128-contraction
        matmul via operand stacking (2 matmuls total). Stage 2 runs 3
        passes, emitted pass-major across bands so LDWEIGHTS of one band
        overlaps the matmul of another (different PE row groups).
  f32:  plain fp32 matmuls (4 cyc/col).
  f32r: float32r (TF32-like, ~4e-4 rel err) - fast but reduced precision.

Sharding: 384 padded cells split 48 per core (identical SPMD program,
per-core S/A1 data). Output layout per core: [t1, group, band, j, t2]; host
transposes to [k, t1, t2] and reassembles the 8-tuple.
"""
import os
import sys
import types

if "/opt/trn_rl_repo" not in sys.path:
    sys.path.insert(0, "/opt/trn_rl_repo")

import numpy as np

L = 7
LSIZE = (L + 1) ** 2          # 64
T = 128
NCORES = 8
SLOT_P = 32                   # partitions per slot band
BANDS = 4                     # bands (cells) per group
GROUP_CHUNKS = 4              # chunks per group == slots per cell

# ------------------------------------------------------------ pair tables
PAIRS = []                    # full table: (l1, l2, lo, hi, rows, row_offset)
_off = 0
for _l1 in range(L + 1):
    for _l2 in range(L + 1):
        _lo, _hi = abs(_l1 - _l2), min(_l1 + _l2, L)
        _rows = (_hi + 1) ** 2 - _lo * _lo
        PAIRS.append((_l1, _l2, _lo, _hi, _rows, _off))
        _off += _rows
TOTAL_ROWS = _off             # 2416

UP_PAIRS = [i for i, p in enumerate(PAIRS) if p[0] <= p[1]]   # 36 pair idxs

# global cell list: (full_pair_idx, k_base); one pair per cell, 4 k's
CELL_TABLE = []
for _pi in UP_PAIRS:
    _l1, _l2, _lo, _hi, _rows, _po = PAIRS[_pi]
    for _kb in range(_lo * _lo, (_hi + 1) ** 2, GROUP_CHUNKS):
        CELL_TABLE.append((_pi, _kb))
N_CELLS = len(CELL_TABLE)     # 356

CELLS_PER_CORE = -(-N_CELLS // NCORES)         # 45
GROUPS = -(-CELLS_PER_CORE // BANDS)           # 12 (last group partial)
GROUP_BANDS = [BANDS] * (GROUPS - 1) +     [CELLS_PER_CORE - BANDS * (GROUPS - 1)]    # [4]*11 + [1]
CHUNKS = GROUPS * GROUP_CHUNKS                 # 48
SLOTS = CELLS_PER_CORE * GROUP_CHUNKS          # 180 per core
ST_COLS = CHUNKS * T                           # 6144
A1_COLS = GROUPS * T                           # 1536
GROUP_OUT = [512 * nb for nb in GROUP_BANDS]   # out cols per group
GROUP_OUT_OFF = [sum(GROUP_OUT[:g]) for g in range(GROUPS)]
OUT_COLS = sum(GROUP_OUT)                      # 23040

_NC = None
_NC_MODE = None
LAST_EXEC_TIME_NS = None


def _build_nc(mode):
    import concourse.bacc as bacc
    import concourse.mybir as mybir
    import concourse.tile as tile

    F32 = mybir.dt.float32
    split = mode == "f16x2"
    WDT = {"f32": F32, "f32r": mybir.dt.float32r,
           "f16x2": mybir.dt.float16}[mode]
    ST_P = 128 if split else 64        # stb rows: [Sh; Sl] stacked when split
    A1W = (2 if split else 1) * A1_COLS

    nc = bacc.Bacc("TRN2", target_bir_lowering=False, debug=False, num_devices=1)
    d_stb = nc.dram_tensor("stb", [ST_P, ST_COLS], WDT, kind="ExternalInput")
    d_cla = nc.dram_tensor("cla", [LSIZE, T], WDT, kind="ExternalInput")
    d_clb = (nc.dram_tensor("clb", [128, T], WDT, kind="ExternalInput")
             if split else None)
    d_a1 = nc.dram_tensor("a1b", [128, A1W], WDT, kind="ExternalInput")
    d_out = nc.dram_tensor("o", [128, OUT_COLS], F32, kind="ExternalOutput")

    Copy = mybir.ActivationFunctionType.Copy

    with tile.TileContext(nc) as tc:
        with (
            tc.tile_pool(name="big", bufs=1) as big,
            tc.tile_pool(name="wpool", bufs=3) as wpool,
            tc.tile_pool(name="stage", bufs=3) as stage,
            tc.tile_pool(name="wps", bufs=3, space="PSUM") as wps_pool,
            tc.tile_pool(name="obank", bufs=4, space="PSUM") as obank,
        ):
            cla_sb = big.tile([LSIZE, T], WDT, name="cla_sb")
            nc.sync.dma_start(cla_sb[:], d_cla[:])
            if split:
                clb_sb = big.tile([128, T], WDT, name="clb_sb")
                nc.sync.dma_start(clb_sb[:], d_clb[:])
            NQ = 4
            stq = ST_COLS // NQ
            st_sbs = [big.tile([ST_P, stq], WDT, name=f"st_sb{q}")
                      for q in range(NQ)]
            a1_sb = big.tile([128, A1W], WDT, name="a1_sb")
            nc.sync.dma_start(st_sbs[0][:], d_stb[:, 0:stq])
            # gate the remaining input DMAs behind st quarter 0 so its DMA
            # gets the full bandwidth and stage 1 starts early (the 1-elem
            # copies create WAW deps that order the dma_starts)
            nc.vector.tensor_copy(st_sbs[1][0:1, 0:1], st_sbs[0][0:1, 0:1])
            nc.vector.tensor_copy(a1_sb[0:1, 0:1], st_sbs[0][0:1, 0:1])
            nc.sync.dma_start(st_sbs[1][:], d_stb[:, stq : 2 * stq])
            nc.sync.dma_start(a1_sb[:], d_a1[:])
            for q in (2, 3):
                nc.vector.tensor_copy(st_sbs[q][0:1, 0:1], st_sbs[1][0:1, 0:1])
                nc.sync.dma_start(st_sbs[q][:], d_stb[:, stq * q : stq * (q + 1)])

            w_tiles = [None] * GROUPS     # per group: [wt_h] or [wt_h, wt_l]

            def s1_tiles(g):
                w_tiles[g] = [wpool.tile([128, GROUP_CHUNKS * T], WDT,
                                         name=f"w_{g}_{h}", tag=f"w{h}")
                              for h in range(2 if split else 1)]
                return [None] * GROUP_CHUNKS

            def s1_mm(g, j, wpss):
                c = GROUP_CHUNKS * g + j
                q, qc = divmod(c, CHUNKS // NQ)
                wps = wps_pool.tile([128, T], mybir.dt.float32,
                                    name=f"wps_{c}", tag="wps")
                if split:
                    nc.tensor.matmul(wps[:],
                                     st_sbs[q][0:64, T * qc : T * (qc + 1)],
                                     cla_sb[:], start=True, stop=False)
                    nc.tensor.matmul(wps[:],
                                     st_sbs[q][:, T * qc : T * (qc + 1)],
                                     clb_sb[:], start=False, stop=True)
                else:
                    nc.tensor.matmul(wps[:],
                                     st_sbs[q][:, T * qc : T * (qc + 1)],
                                     cla_sb[:], start=True, stop=True)
                wpss[j] = wps

            def s1_copy(g, j, wpss):
                wps = wpss[j]
                dst_h = w_tiles[g][0][:, T * j : T * (j + 1)]
                if split:
                    nc.scalar.activation(dst_h, wps[:], Copy)
                    nc.vector.tensor_sub(w_tiles[g][1][:, T * j : T * (j + 1)],
                                         wps[:], dst_h)
                elif j % 2 == 0:
                    nc.vector.tensor_copy(dst_h, wps[:])
                else:
                    nc.scalar.activation(dst_h, wps[:], Copy)

            def s2_banks(g):
                return [obank.tile([128, 512], mybir.dt.float32,
                                   name=f"bank_{g}_{b}", tag="obank")
                        for b in range(GROUP_BANDS[g])]

            def s2_pass(g, banks, n, npass):
                # pass-major emission: LDWEIGHTS of band b overlaps the
                # in-flight matmul of band b-1 (different PE row groups)
                if split:
                    ha, hw = [(0, 0), (0, 1), (1, 0)][n]
                else:
                    ha, hw = 0, 0
                for b in range(GROUP_BANDS[g]):
                    p = SLOT_P * b
                    a1ap = a1_sb[p : p + SLOT_P,
                                 A1_COLS * ha + T * g : A1_COLS * ha + T * (g + 1)]
                    nc.tensor.matmul(
                        banks[b][:], a1ap, w_tiles[g][hw][p : p + SLOT_P, :],
                        start=(n == 0), stop=(n == npass - 1),
                        tile_position=(p, 0),
                    )

            def s2_copy(g, banks, o_tile):
                for b in range(GROUP_BANDS[g]):
                    dst = o_tile[:, 512 * b : 512 * (b + 1)]
                    if b % 2 == 0:
                        nc.vector.tensor_copy(dst, banks[b][:])
                    else:
                        nc.scalar.activation(dst, banks[b][:], Copy)

            npass = 3 if split else 1
            # prologue: stage-1 for group 0
            wpss = s1_tiles(0)
            for j in range(GROUP_CHUNKS):
                s1_mm(0, j, wpss)
                s1_copy(0, j, wpss)
            for g in range(GROUPS):
                o_tile = stage.tile([128, GROUP_OUT[g]],
                                    mybir.dt.float32, name=f"o_{g}", tag="o")
                banks = s2_banks(g)
                if g + 1 < GROUPS:
                    # interleave next group's stage-1 matmuls between passes
                    wpss = s1_tiles(g + 1)
                    for n in range(npass):
                        s1_mm(g + 1, n, wpss)
                        s1_copy(g + 1, n, wpss)
                        s2_pass(g, banks, n, npass)
                    for j in range(npass, GROUP_CHUNKS):
                        s1_mm(g + 1, j, wpss)
                        s1_copy(g + 1, j, wpss)
                else:
                    for n in range(npass):
                        s2_pass(g, banks, n, npass)
                s2_copy(g, banks, o_tile)
                nc.sync.dma_start(
                    d_out[:, GROUP_OUT_OFF[g] : GROUP_OUT_OFF[g] + GROUP_OUT[g]],
                    o_tile[:])
    nc.compile()
    return nc


def _install_profile_hook():
    try:
        import antenv
        from concourse import bass_utils
        if "antenv.axon_hooks" not in sys.modules:
            mod = types.ModuleType("antenv.axon_hooks")
            mod._hook = None
            mod.set_axon_ntff_profile_hook = lambda h: setattr(mod, "_hook", h)
            mod.get_axon_ntff_profile_hook = lambda: mod._hook
            sys.modules["antenv.axon_hooks"] = mod
            antenv.axon_hooks = mod
        from trn_agent_boot.trn_boot import _ntff_profile_via_ctypes
        sys.modules["antenv.axon_hooks"].set_axon_ntff_profile_hook(
            _ntff_profile_via_ctypes("/opt/axon/libaxon_pjrt.so"))
        bass_utils.upload_artifacts = lambda tmpdir: f"local:{tmpdir}"
    except Exception as e:
        print(f"kernel: profile hook unavailable ({e})", file=sys.stderr)


def _build_core_inputs(clms, C, core):
    """Dense fp32 s_t [64, ST_COLS] and a1s [128, A1_COLS] for one core."""
    s_t = np.zeros((LSIZE, ST_COLS), np.float32)
    a1s = np.zeros((128, A1_COLS), np.float32)
    base = CELLS_PER_CORE * core
    for c_loc in range(CELLS_PER_CORE):
        ci = base + c_loc
        if ci >= N_CELLS:
            break
        pi, kb = CELL_TABLE[ci]
        l1, l2, lo, hi, rows, po = PAIRS[pi]
        d1, d2 = 2 * l1 + 1, 2 * l2 + 1
        g, b = divmod(c_loc, BANDS)
        a1s[SLOT_P * b : SLOT_P * b + d1, T * g : T * (g + 1)] = \
            clms[l1 * l1 : l1 * l1 + d1, :]
        kmax = (hi + 1) ** 2
        for j in range(GROUP_CHUNKS):
            k = kb + j
            if k >= kmax:
                break
            col = T * (GROUP_CHUNKS * g + j) + SLOT_P * b
            blk = C[k, l1 * l1 : l1 * l1 + d1, l2 * l2 : l2 * l2 + d2]
            s_t[l2 * l2 : l2 * l2 + d2, col : col + d1] = blk.T
    return s_t, a1s


def _split16(x):
    hi = x.astype(np.float16)
    lo = (x - hi.astype(np.float32)).astype(np.float16)
    return hi, lo


def kernel(clms, C):
    global _NC, _NC_MODE, LAST_EXEC_TIME_NS
    from concourse.bass_utils import run_bass_kernel_spmd

    trace = os.environ.get("BASS_TRACE", "0") == "1"
    mode = os.environ.get("KERNEL_MODE", "f16x2")
    if trace:
        _install_profile_hook()

    clms = np.ascontiguousarray(np.asarray(clms, dtype=np.float32))
    C = np.ascontiguousarray(np.asarray(C, dtype=np.float32))

    if _NC is None or _NC_MODE != mode:
        _NC = _build_nc(mode)
        _NC_MODE = mode

    in_maps = []
    for core in range(NCORES):
        s_t, a1s = _build_core_inputs(clms, C, core)
        if mode == "f16x2":
            sh, sl = _split16(s_t)
            ah, al = _split16(a1s)
            ch, cl = _split16(clms)
            in_maps.append({
                "stb": np.concatenate([sh, sl], axis=0),
                "cla": ch,
                "clb": np.concatenate([cl, ch], axis=0),
                "a1b": np.concatenate([ah, al], axis=1),
            })
        else:
            in_maps.append({"stb": s_t, "cla": clms, "a1b": a1s})

    res = run_bass_kernel_spmd(_NC, in_maps, list(range(NCORES)), trace=trace)
    LAST_EXEC_TIME_NS = res.exec_time_ns

    # ---------------- host reassembly ----------------
    G = np.empty((TOTAL_ROWS, T, T), np.float32)
    for core in range(NCORES):
        o = res.results[core]["o"]          # [128, OUT_COLS]
        base = CELLS_PER_CORE * core
        for c_loc in range(CELLS_PER_CORE):
            ci = base + c_loc
            if ci >= N_CELLS:
                break
            pi, kb = CELL_TABLE[ci]
            l1, l2, lo, hi, rows, po = PAIRS[pi]
            kmax = (hi + 1) ** 2
            g, b = divmod(c_loc, BANDS)
            for j in range(GROUP_CHUNKS):
                k = kb + j
                if k >= kmax:
                    break
                cb = GROUP_OUT_OFF[g] + 512 * b + 128 * j
                G[po + (k - lo * lo)] = o[:, cb : cb + T]

    # mirror lower pairs (l1 > l2) from upper: OUT = sign * OUT_upper^T
    ls = np.arange(LSIZE)
    l_of_k = np.floor(np.sqrt(ls)).astype(np.int64)
    for pi, (l1, l2, lo, hi, rows, po) in enumerate(PAIRS):
        if l1 <= l2:
            continue
        po_u = PAIRS[8 * l2 + l1][5]
        ks = np.arange(lo * lo, (hi + 1) ** 2)
        sign = ((-1.0) ** (l1 + l2 - l_of_k[ks])).astype(np.float32)
        G[po : po + rows] = sign[:, None, None] * \
            G[po_u : po_u + rows].transpose(0, 2, 1)

    Gf = G.reshape(TOTAL_ROWS, T * T)
    out = []
    for l in range(L + 1):
        blocks = []
        for (l1, l2, lo, hi, rows, po) in PAIRS:
            if lo <= l <= hi:
                r0 = po + (l * l - lo * lo)
                blocks.append(Gf[r0 : r0 + 2 * l + 1, :])
        out.append(np.concatenate(blocks, axis=1))
    return tuple(out)
